# revision 10
# baseline (speedup 1.0000x reference)
"""Sparse (shot-local + shared-global) attention on 8 Trainium2 NeuronCores.

Problem: B=2, S_TOT=4096, HD=1024 with H=16 heads (d=64), num_shots=4
(L=1024 tokens per shot), global pool = first 64 tokens of each shot
(G=256), shared by all shots of the same batch element.

Sharding: the 32 (batch, head) pairs are split 4-per-core across 8 cores
(data + head parallel). Each (b,h,shot) block is independent attention of
shape q[1024,64] against k/v[1024+256,64].

Per-core kernel (per pair, shot, 512-wide q-chunk):
  S^T[k,q]   = kT_tile.T @ qT            (PE, k tokens on partitions)
  P^T        = exp(S^T * 1/8)            (ACT, groups of 3 PSUM banks)
  [o^T; Z]   = [v | 1].T @ P^T           (PE, accumulated over k tiles)
  o^T        = o^T * (1/Z broadcast)     (DVE recip + GpSimd bcast + DVE mul)
Softmax max-subtraction is skipped: logits are ~N(0,1), |logit| < ~6, exp
is safely in range.

The S matmuls contract over d=64 (half the PE rows), so pairs of k-tiles
are packed into the two 64-row strips of the PE array (tile_position
(0,0) / (64,0)) and run concurrently: host packs even k-tiles into SBUF
partitions 0-63 and odd k-tiles into partitions 64-127, with the q tile
duplicated into both halves.

Host packs q/k into [d, tokens] (transposed) layout and v into [128, t, 65]
tiles with a ones column (the ones column makes the PV matmul emit the
softmax denominator Z as PSUM row 64). Host transposes o^T back at gather.
"""

import sys

sys.path.insert(0, "/opt/trn_rl_repo")

import ml_dtypes
import numpy as np

import concourse.bass as bass  # noqa: F401  (registers AP machinery)
import concourse.mybir as mybir
import concourse.tile as tile
from concourse import bacc
from concourse.bass_utils import run_bass_kernel_spmd

B, S_TOT, HD = 2, 4096, 1024
H, NSHOT, PER_G = 16, 4, 64
D = HD // H            # 64 head dim
L = S_TOT // NSHOT     # 1024 shot length
G = NSHOT * PER_G      # 256 global pool tokens
NCORES = 8
PAIRS = (B * H) // NCORES   # 4 (b,h) pairs per core
QC = 512                    # q chunk width (PSUM bank)
NQC = L // QC               # 2
NKT_LOC = L // 128          # 8 local k tiles per shot
NKT = NKT_LOC + G // 128    # 10 k tiles (slots) total per shot
NROUND = NKT // 2           # 5 row-packed S rounds per (shot, qc)
SCALE = 1.0 / float(np.sqrt(D))
# slot -> (exp group, offset): groups of 3 slots (3 PSUM banks) + 1 tail
GROUP_OF = {j: (j // 3, j % 3) for j in range(NKT)}
NGROUP = 4
GROUP_SLOTS = [[j for j in range(NKT) if GROUP_OF[j][0] == g] for g in range(NGROUP)]

MM_DT = "bfloat16"   # matmul operand dtype ("bfloat16" | "float16")

_NC = None


def build_program():
    """Build + compile the per-core Bass program (identical on all cores)."""
    global _NC
    if _NC is not None:
        return _NC
    f32 = mybir.dt.float32
    mdt = getattr(mybir.dt, MM_DT)
    Exp = mybir.ActivationFunctionType.Exp

    nc = bacc.Bacc("TRN2", target_bir_lowering=False, debug=True)
    # qT2: q transposed, duplicated into both 64-partition halves.
    qT2_d = nc.dram_tensor("qT2", [128, PAIRS, S_TOT], mdt, kind="ExternalInput")
    # kT2: even k-tiles in partitions 0-63, odd k-tiles in 64-127.
    kT2_d = nc.dram_tensor("kT2", [128, PAIRS, S_TOT // 2], mdt,
                           kind="ExternalInput")
    kgT2_d = nc.dram_tensor("kgT2", [128, PAIRS, G // 2], mdt,
                            kind="ExternalInput")
    v65_d = nc.dram_tensor("v65", [128, PAIRS, NKT_LOC * NSHOT, 65], mdt,
                           kind="ExternalInput")
    vg65_d = nc.dram_tensor("vg65", [128, PAIRS, G // 128, 65], mdt,
                            kind="ExternalInput")
    oT_d = nc.dram_tensor("oT", [D, PAIRS, S_TOT], f32, kind="ExternalOutput")

    with tile.TileContext(nc) as tc:
        with (
            tc.tile_pool(name="inp", bufs=2) as inp_pool,
            tc.tile_pool(name="work", bufs=3) as work_pool,
            tc.tile_pool(name="ps_s", bufs=2, space="PSUM") as ps_pool,
            tc.tile_pool(name="ps_o", bufs=2, space="PSUM") as po_pool,
        ):
            class Unit:
                """One (pair, shot, q-chunk) attention block's emitters."""

                def __init__(self, sbufs, s, qc):
                    self.sb = sbufs
                    self.s = s
                    self.qcol = s * L + qc * QC
                    self.po = po_pool.tile([65, QC], f32, tag="po", name="po")
                    self.gt = [None] * NGROUP
                    self.ex = [None] * NGROUP

                def S_round(self, r):
                    for half in (0, 1):
                        slot = 2 * r + half
                        g, off = GROUP_OF[slot]
                        if self.gt[g] is None:
                            n = len(GROUP_SLOTS[g])
                            self.gt[g] = ps_pool.tile([128, QC * n], f32,
                                                      tag="ps", name=f"ps{g}")
                        if r < NROUND - 1:  # local rounds 0..3
                            k_lhs = self.sb["kT2"][half * 64:(half + 1) * 64,
                                                   self.s * (L // 2) + r * 128:
                                                   self.s * (L // 2) + (r + 1) * 128]
                        else:               # global round
                            k_lhs = self.sb["kgT2"][half * 64:(half + 1) * 64, :]
                        nc.tensor.matmul(
                            self.gt[g][:, off * QC:(off + 1) * QC],
                            k_lhs,
                            self.sb["qT2"][half * 64:(half + 1) * 64,
                                           self.qcol:self.qcol + QC],
                            start=True, stop=True,
                            tile_position=(half * 64, 0),
                        )

                def E(self, g):
                    n = len(GROUP_SLOTS[g])
                    expT = work_pool.tile([128, QC * n], mdt, tag="expT",
                                          name=f"expT{g}", bufs=4)
                    nc.scalar.activation(expT[:], self.gt[g][:], Exp, scale=SCALE)
                    self.ex[g] = expT

                def PV(self, g):
                    for off, slot in enumerate(GROUP_SLOTS[g]):
                        if slot < NKT_LOC:
                            v_lhs = self.sb["v65"][:, self.s * NKT_LOC + slot, :]
                        else:
                            v_lhs = self.sb["vg65"][:, slot - NKT_LOC, :]
                        nc.tensor.matmul(
                            self.po[:], v_lhs,
                            self.ex[g][:, off * QC:(off + 1) * QC],
                            start=(slot == 0), stop=(slot == NKT - 1),
                        )

                def EPI(self):
                    zsb = work_pool.tile([1, QC], f32, tag="zsb")
                    nc.vector.tensor_copy(zsb[:], self.po[64:65, :])
                    zr = work_pool.tile([1, QC], f32, tag="zr")
                    nc.vector.reciprocal_approx_fast(zr[:], zsb[:])
                    zb = work_pool.tile([64, QC], f32, tag="zb")
                    nc.gpsimd.partition_broadcast(zb[:], zr[:])
                    oT_sb = work_pool.tile([64, QC], f32, tag="oT")
                    nc.vector.tensor_mul(oT_sb[:], self.po[0:64, :], zb[:])
                    nc.sync.dma_start(
                        oT_d[:, self.sb["p"], self.qcol:self.qcol + QC], oT_sb[:])

            def load_pair(p):
                qT2_sb = inp_pool.tile([128, S_TOT], mdt, tag="qT", name="qT2_sb")
                nc.sync.dma_start(qT2_sb[:], qT2_d[:, p, :])
                kT2_sb = inp_pool.tile([128, S_TOT // 2], mdt, tag="kT",
                                       name="kT2_sb")
                nc.sync.dma_start(kT2_sb[:], kT2_d[:, p, :])
                kgT2_sb = inp_pool.tile([128, G // 2], mdt, tag="kgT",
                                        name="kgT2_sb")
                nc.sync.dma_start(kgT2_sb[:], kgT2_d[:, p, :])
                v65_sb = inp_pool.tile([128, NKT_LOC * NSHOT, 65], mdt,
                                       tag="v65", name="v65_sb")
                nc.sync.dma_start(v65_sb[:], v65_d[:, p, :, :])
                vg65_sb = inp_pool.tile([128, G // 128, 65], mdt, tag="vg65",
                                        name="vg65_sb")
                nc.sync.dma_start(vg65_sb[:], vg65_d[:, p, :, :])
                return {"p": p, "qT2": qT2_sb, "kT2": kT2_sb, "kgT2": kgT2_sb,
                        "v65": v65_sb, "vg65": vg65_sb}

            # Software-pipelined emission: unit U's last exp group, PV tail and
            # epilogue are emitted after unit U+1's first S rounds so the PE
            # always has independent work queued while ACT runs exp.
            prev = None
            sbufs = None
            for s_p in range(PAIRS):
                sbufs = load_pair(s_p)
                for s_s in range(NSHOT):
                    for s_qc in range(NQC):
                        u = Unit(sbufs, s_s, s_qc)
                        u.S_round(0)
                        u.S_round(1)
                        if prev is not None:
                            prev.E(3)
                            prev.PV(3)
                            prev.EPI()
                        u.E(0)
                        u.S_round(2)
                        u.E(1)
                        u.PV(0)
                        u.S_round(3)
                        u.PV(1)
                        u.S_round(4)
                        u.E(2)
                        u.PV(2)
                        prev = u
            prev.E(3)
            prev.PV(3)
            prev.EPI()
    nc.compile()
    _NC = nc
    return nc


def pack_inputs(q, k, v):
    """Shard + relayout full inputs into per-core input maps."""
    ndt = ml_dtypes.bfloat16 if MM_DT == "bfloat16" else np.float16
    q5 = np.ascontiguousarray(q).reshape(B, S_TOT, H, D)
    k5 = np.ascontiguousarray(k).reshape(B, S_TOT, H, D)
    v5 = np.ascontiguousarray(v).reshape(B, S_TOT, H, D)
    gidx = (np.arange(NSHOT)[:, None] * L + np.arange(PER_G)[None, :]).reshape(-1)

    in_maps = []
    for c in range(NCORES):
        qT2 = np.empty((128, PAIRS, S_TOT), ndt)
        kT2 = np.empty((128, PAIRS, S_TOT // 2), ndt)
        kgT2 = np.empty((128, PAIRS, G // 2), ndt)
        v65 = np.ones((128, PAIRS, NKT_LOC * NSHOT, 65), ndt)
        vg65 = np.ones((128, PAIRS, G // 128, 65), ndt)
        for p in range(PAIRS):
            pair = c * PAIRS + p
            b, h = divmod(pair, H)
            qT = q5[b, :, h, :].T                      # [64, S_TOT]
            qT2[0:64, p, :] = qT
            qT2[64:128, p, :] = qT
            # k tiles: [64, NSHOT, 8, 128] -> even tiles top, odd bottom
            kt = k5[b, :, h, :].T.reshape(D, NSHOT, NKT_LOC, 128)
            kT2[0:64, p, :] = kt[:, :, 0::2, :].reshape(D, S_TOT // 2)
            kT2[64:128, p, :] = kt[:, :, 1::2, :].reshape(D, S_TOT // 2)
            kg = k5[b, gidx, h, :].T                   # [64, 256]
            kgT2[0:64, p, :] = kg[:, 0:128]
            kgT2[64:128, p, :] = kg[:, 128:256]
            # [S_TOT, 64] -> [n_tiles, 128, 64] -> [128, n_tiles, 64]
            v65[:, p, :, :64] = v5[b, :, h, :].reshape(-1, 128, D).transpose(1, 0, 2)
            vg65[:, p, :, :64] = v5[b, gidx, h, :].reshape(-1, 128, D).transpose(1, 0, 2)
        in_maps.append({"qT2": qT2, "kT2": kT2, "kgT2": kgT2,
                        "v65": v65, "vg65": vg65})
    return in_maps


def unpack_outputs(results):
    """Per-core oT [D, PAIRS, S_TOT] -> full [B, S_TOT, HD]."""
    out5 = np.empty((B, S_TOT, H, D), np.float32)
    for c in range(NCORES):
        oT = results[c]["oT"]
        for p in range(PAIRS):
            b, h = divmod(c * PAIRS + p, H)
            out5[b, :, h, :] = oT[:, p, :].T
    return out5.reshape(B, S_TOT, HD)


def kernel(q, k, v, num_heads, num_shots, per_g):
    assert int(num_heads) == H and int(num_shots) == NSHOT and int(per_g) == PER_G
    nc = build_program()
    in_maps = pack_inputs(np.asarray(q), np.asarray(k), np.asarray(v))
    res = run_bass_kernel_spmd(nc, in_maps, list(range(NCORES)))
    return unpack_outputs(res.results)


# revision 11
# speedup vs baseline: 1.1837x; 1.1837x over previous
"""Sparse (shot-local + shared-global) attention on 8 Trainium2 NeuronCores.

Problem: B=2, S_TOT=4096, HD=1024 with H=16 heads (d=64), num_shots=4
(L=1024 tokens per shot), global pool = first 64 tokens of each shot
(G=256), shared by all shots of the same batch element.

Sharding: the 32 (batch, head) pairs are split 4-per-core across 8 cores
(data + head parallel). Each (b,h,shot) block is independent attention of
shape q[1024,64] against k/v[1024+256,64].

Per-core kernel (per pair, shot, 512-wide q-chunk):
  S^T[k,q]   = kT_tile.T @ qT            (PE, k tokens on partitions)
  P^T        = exp(S^T * 1/8)            (ACT, groups of 3 PSUM banks)
  [o^T; Z]   = [v | 1].T @ P^T           (PE, accumulated over k tiles)
  o^T        = o^T * (1/Z broadcast)     (DVE recip + GpSimd bcast + DVE mul)
Softmax max-subtraction is skipped: logits are ~N(0,1), |logit| < ~6, exp
is safely in range.

The S matmuls contract over d=64 (half the PE rows), so pairs of k-tiles
are packed into the two 64-row strips of the PE array (tile_position
(0,0) / (64,0)) and run concurrently: host packs even k-tiles into SBUF
partitions 0-63 and odd k-tiles into partitions 64-127, with the q tile
duplicated into both halves.

Host packs q/k into [d, tokens] (transposed) layout and v into [128, t, 65]
tiles with a ones column (the ones column makes the PV matmul emit the
softmax denominator Z as PSUM row 64). Host transposes o^T back at gather.
"""

import sys

sys.path.insert(0, "/opt/trn_rl_repo")

import ml_dtypes
import numpy as np

import concourse.bass as bass  # noqa: F401  (registers AP machinery)
import concourse.mybir as mybir
import concourse.tile as tile
from concourse import bacc
from concourse.bass_utils import run_bass_kernel_spmd

B, S_TOT, HD = 2, 4096, 1024
H, NSHOT, PER_G = 16, 4, 64
D = HD // H            # 64 head dim
L = S_TOT // NSHOT     # 1024 shot length
G = NSHOT * PER_G      # 256 global pool tokens
NCORES = 8
PAIRS = (B * H) // NCORES   # 4 (b,h) pairs per core
QC = 512                    # q chunk width (PSUM bank)
NQC = L // QC               # 2
NKT_LOC = L // 128          # 8 local k tiles per shot
NKT = NKT_LOC + G // 128    # 10 k tiles (slots) total per shot
NROUND = NKT // 2           # 5 row-packed S rounds per (shot, qc)
SCALE = 1.0 / float(np.sqrt(D))
# slot -> (exp group, offset): groups of 3 slots (3 PSUM banks) + 1 tail
GROUP_OF = {j: (j // 3, j % 3) for j in range(NKT)}
NGROUP = 4
GROUP_SLOTS = [[j for j in range(NKT) if GROUP_OF[j][0] == g] for g in range(NGROUP)]

MM_DT = "float16"   # matmul operand dtype ("bfloat16" | "float16")

_NC = None


def build_program():
    """Build + compile the per-core Bass program (identical on all cores)."""
    global _NC
    if _NC is not None:
        return _NC
    f32 = mybir.dt.float32
    mdt = getattr(mybir.dt, MM_DT)
    Exp = mybir.ActivationFunctionType.Exp

    nc = bacc.Bacc("TRN2", target_bir_lowering=False, debug=True)
    # qT2: q transposed, duplicated into both 64-partition halves.
    qT2_d = nc.dram_tensor("qT2", [128, PAIRS, S_TOT], mdt, kind="ExternalInput")
    # kT2: even k-tiles in partitions 0-63, odd k-tiles in 64-127.
    kT2_d = nc.dram_tensor("kT2", [128, PAIRS, S_TOT // 2], mdt,
                           kind="ExternalInput")
    kgT2_d = nc.dram_tensor("kgT2", [128, PAIRS, G // 2], mdt,
                            kind="ExternalInput")
    v65_d = nc.dram_tensor("v65", [128, PAIRS, NKT_LOC * NSHOT, 65], mdt,
                           kind="ExternalInput")
    vg65_d = nc.dram_tensor("vg65", [128, PAIRS, G // 128, 65], mdt,
                            kind="ExternalInput")
    oT_d = nc.dram_tensor("oT", [D, PAIRS, S_TOT], f32, kind="ExternalOutput")

    with tile.TileContext(nc) as tc:
        with (
            tc.tile_pool(name="inp", bufs=2) as inp_pool,
            tc.tile_pool(name="work", bufs=3) as work_pool,
            tc.tile_pool(name="ps_s", bufs=2, space="PSUM") as ps_pool,
            tc.tile_pool(name="ps_o", bufs=2, space="PSUM") as po_pool,
        ):
            class Unit:
                """One (pair, shot, q-chunk) attention block's emitters."""

                def __init__(self, sbufs, s, qc):
                    self.sb = sbufs
                    self.s = s
                    self.qcol = s * L + qc * QC
                    self.po = po_pool.tile([65, QC], f32, tag="po", name="po")
                    self.gt = [None] * NGROUP
                    self.ex = [None] * NGROUP

                def S_round(self, r):
                    for half in (0, 1):
                        slot = 2 * r + half
                        g, off = GROUP_OF[slot]
                        if self.gt[g] is None:
                            n = len(GROUP_SLOTS[g])
                            self.gt[g] = ps_pool.tile([128, QC * n], f32,
                                                      tag="ps", name=f"ps{g}")
                        if r < NROUND - 1:  # local rounds 0..3
                            k_lhs = self.sb["kT2"][half * 64:(half + 1) * 64,
                                                   self.s * (L // 2) + r * 128:
                                                   self.s * (L // 2) + (r + 1) * 128]
                        else:               # global round
                            k_lhs = self.sb["kgT2"][half * 64:(half + 1) * 64, :]
                        nc.tensor.matmul(
                            self.gt[g][:, off * QC:(off + 1) * QC],
                            k_lhs,
                            self.sb["qT2"][half * 64:(half + 1) * 64,
                                           self.qcol:self.qcol + QC],
                            start=True, stop=True,
                            tile_position=(half * 64, 0),
                        )

                def E(self, g):
                    n = len(GROUP_SLOTS[g])
                    expT = work_pool.tile([128, QC * n], mdt, tag="expT",
                                          name=f"expT{g}", bufs=4)
                    nc.scalar.activation(expT[:], self.gt[g][:], Exp, scale=SCALE)
                    self.ex[g] = expT

                def PV(self, g):
                    for off, slot in enumerate(GROUP_SLOTS[g]):
                        if slot < NKT_LOC:
                            v_lhs = self.sb["v65"][:, self.s * NKT_LOC + slot, :]
                        else:
                            v_lhs = self.sb["vg65"][:, slot - NKT_LOC, :]
                        nc.tensor.matmul(
                            self.po[:], v_lhs,
                            self.ex[g][:, off * QC:(off + 1) * QC],
                            start=(slot == 0), stop=(slot == NKT - 1),
                        )

                def EPI(self):
                    zsb = work_pool.tile([1, QC], f32, tag="zsb")
                    nc.vector.tensor_copy(zsb[:], self.po[64:65, :])
                    zr = work_pool.tile([1, QC], f32, tag="zr")
                    nc.vector.reciprocal_approx_fast(zr[:], zsb[:])
                    zb = work_pool.tile([64, QC], f32, tag="zb")
                    nc.gpsimd.partition_broadcast(zb[:], zr[:])
                    oT_sb = work_pool.tile([64, QC], f32, tag="oT")
                    nc.vector.tensor_mul(oT_sb[:], self.po[0:64, :], zb[:])
                    nc.sync.dma_start(
                        oT_d[:, self.sb["p"], self.qcol:self.qcol + QC], oT_sb[:])

            def load_pair(p):
                qT2_sb = inp_pool.tile([128, S_TOT], mdt, tag="qT", name="qT2_sb")
                nc.sync.dma_start(qT2_sb[:], qT2_d[:, p, :])
                kT2_sb = inp_pool.tile([128, S_TOT // 2], mdt, tag="kT",
                                       name="kT2_sb")
                nc.sync.dma_start(kT2_sb[:], kT2_d[:, p, :])
                kgT2_sb = inp_pool.tile([128, G // 2], mdt, tag="kgT",
                                        name="kgT2_sb")
                nc.sync.dma_start(kgT2_sb[:], kgT2_d[:, p, :])
                v65_sb = inp_pool.tile([128, NKT_LOC * NSHOT, 65], mdt,
                                       tag="v65", name="v65_sb")
                nc.sync.dma_start(v65_sb[:], v65_d[:, p, :, :])
                vg65_sb = inp_pool.tile([128, G // 128, 65], mdt, tag="vg65",
                                        name="vg65_sb")
                nc.sync.dma_start(vg65_sb[:], vg65_d[:, p, :, :])
                return {"p": p, "qT2": qT2_sb, "kT2": kT2_sb, "kgT2": kgT2_sb,
                        "v65": v65_sb, "vg65": vg65_sb}

            # Software-pipelined emission: unit U's last exp group, PV tail and
            # epilogue are emitted after unit U+1's first S rounds so the PE
            # always has independent work queued while ACT runs exp.
            prev = None
            sbufs = None
            for s_p in range(PAIRS):
                sbufs = load_pair(s_p)
                for s_s in range(NSHOT):
                    for s_qc in range(NQC):
                        u = Unit(sbufs, s_s, s_qc)
                        u.S_round(0)
                        u.S_round(1)
                        if prev is not None:
                            prev.E(3)
                            prev.PV(3)
                            prev.EPI()
                        u.E(0)
                        u.S_round(2)
                        u.E(1)
                        u.PV(0)
                        u.S_round(3)
                        u.PV(1)
                        u.S_round(4)
                        u.E(2)
                        u.PV(2)
                        prev = u
            prev.E(3)
            prev.PV(3)
            prev.EPI()
    nc.compile()
    _NC = nc
    return nc


def pack_inputs(q, k, v):
    """Shard + relayout full inputs into per-core input maps."""
    ndt = ml_dtypes.bfloat16 if MM_DT == "bfloat16" else np.float16
    q5 = np.ascontiguousarray(q).reshape(B, S_TOT, H, D)
    k5 = np.ascontiguousarray(k).reshape(B, S_TOT, H, D)
    v5 = np.ascontiguousarray(v).reshape(B, S_TOT, H, D)
    gidx = (np.arange(NSHOT)[:, None] * L + np.arange(PER_G)[None, :]).reshape(-1)

    in_maps = []
    for c in range(NCORES):
        qT2 = np.empty((128, PAIRS, S_TOT), ndt)
        kT2 = np.empty((128, PAIRS, S_TOT // 2), ndt)
        kgT2 = np.empty((128, PAIRS, G // 2), ndt)
        v65 = np.ones((128, PAIRS, NKT_LOC * NSHOT, 65), ndt)
        vg65 = np.ones((128, PAIRS, G // 128, 65), ndt)
        for p in range(PAIRS):
            pair = c * PAIRS + p
            b, h = divmod(pair, H)
            qT = q5[b, :, h, :].T                      # [64, S_TOT]
            qT2[0:64, p, :] = qT
            qT2[64:128, p, :] = qT
            # k tiles: [64, NSHOT, 8, 128] -> even tiles top, odd bottom
            kt = k5[b, :, h, :].T.reshape(D, NSHOT, NKT_LOC, 128)
            kT2[0:64, p, :] = kt[:, :, 0::2, :].reshape(D, S_TOT // 2)
            kT2[64:128, p, :] = kt[:, :, 1::2, :].reshape(D, S_TOT // 2)
            kg = k5[b, gidx, h, :].T                   # [64, 256]
            kgT2[0:64, p, :] = kg[:, 0:128]
            kgT2[64:128, p, :] = kg[:, 128:256]
            # [S_TOT, 64] -> [n_tiles, 128, 64] -> [128, n_tiles, 64]
            v65[:, p, :, :64] = v5[b, :, h, :].reshape(-1, 128, D).transpose(1, 0, 2)
            vg65[:, p, :, :64] = v5[b, gidx, h, :].reshape(-1, 128, D).transpose(1, 0, 2)
        in_maps.append({"qT2": qT2, "kT2": kT2, "kgT2": kgT2,
                        "v65": v65, "vg65": vg65})
    return in_maps


def unpack_outputs(results):
    """Per-core oT [D, PAIRS, S_TOT] -> full [B, S_TOT, HD]."""
    out5 = np.empty((B, S_TOT, H, D), np.float32)
    for c in range(NCORES):
        oT = results[c]["oT"]
        for p in range(PAIRS):
            b, h = divmod(c * PAIRS + p, H)
            out5[b, :, h, :] = oT[:, p, :].T
    return out5.reshape(B, S_TOT, HD)


def kernel(q, k, v, num_heads, num_shots, per_g):
    assert int(num_heads) == H and int(num_shots) == NSHOT and int(per_g) == PER_G
    nc = build_program()
    in_maps = pack_inputs(np.asarray(q), np.asarray(k), np.asarray(v))
    res = run_bass_kernel_spmd(nc, in_maps, list(range(NCORES)))
    return unpack_outputs(res.results)


# revision 12
# speedup vs baseline: 1.1902x; 1.0056x over previous
"""Sparse (shot-local + shared-global) attention on 8 Trainium2 NeuronCores.

Problem: B=2, S_TOT=4096, HD=1024 with H=16 heads (d=64), num_shots=4
(L=1024 tokens per shot), global pool = first 64 tokens of each shot
(G=256), shared by all shots of the same batch element.

Sharding: the 32 (batch, head) pairs are split 4-per-core across 8 cores
(data + head parallel). Each (b,h,shot) block is independent attention of
shape q[1024,64] against k/v[1024+256,64].

Per-core kernel (per pair, shot, 512-wide q-chunk):
  S^T[k,q]   = kT_tile.T @ qT            (PE, k tokens on partitions)
  P^T        = exp(S^T * 1/8)            (ACT, groups of 3 PSUM banks)
  [o^T; Z]   = [v | 1].T @ P^T           (PE, accumulated over k tiles)
  o^T        = o^T * (1/Z broadcast)     (DVE recip + GpSimd bcast + DVE mul)
Softmax max-subtraction is skipped: logits are ~N(0,1), |logit| < ~6, exp
is safely in range.

The S matmuls contract over d=64 (half the PE rows), so pairs of k-tiles
are packed into the two 64-row strips of the PE array (tile_position
(0,0) / (64,0)) and run concurrently: host packs even k-tiles into SBUF
partitions 0-63 and odd k-tiles into partitions 64-127, with the q tile
duplicated into both halves.

Host packs q/k into [d, tokens] (transposed) layout and v into [128, t, 65]
tiles with a ones column (the ones column makes the PV matmul emit the
softmax denominator Z as PSUM row 64). Host transposes o^T back at gather.
"""

import sys

sys.path.insert(0, "/opt/trn_rl_repo")

import ml_dtypes
import numpy as np

import concourse.bass as bass  # noqa: F401  (registers AP machinery)
import concourse.mybir as mybir
import concourse.tile as tile
from concourse import bacc
from concourse.bass_utils import run_bass_kernel_spmd

B, S_TOT, HD = 2, 4096, 1024
H, NSHOT, PER_G = 16, 4, 64
D = HD // H            # 64 head dim
L = S_TOT // NSHOT     # 1024 shot length
G = NSHOT * PER_G      # 256 global pool tokens
NCORES = 8
PAIRS = (B * H) // NCORES   # 4 (b,h) pairs per core
QC = 512                    # q chunk width (PSUM bank)
NQC = L // QC               # 2
NKT_LOC = L // 128          # 8 local k tiles per shot
NKT = NKT_LOC + G // 128    # 10 k tiles (slots) total per shot
NROUND = NKT // 2           # 5 row-packed S rounds per (shot, qc)
SCALE = 1.0 / float(np.sqrt(D))
# slot -> (exp group, offset): groups of 3 slots (3 PSUM banks) + 1 tail
GROUP_OF = {j: (j // 3, j % 3) for j in range(NKT)}
NGROUP = 4
GROUP_SLOTS = [[j for j in range(NKT) if GROUP_OF[j][0] == g] for g in range(NGROUP)]

MM_DT = "bfloat16"   # matmul operand dtype ("bfloat16" | "float16")

_NC = None


def build_program():
    """Build + compile the per-core Bass program (identical on all cores)."""
    global _NC
    if _NC is not None:
        return _NC
    f32 = mybir.dt.float32
    mdt = getattr(mybir.dt, MM_DT)
    Exp = mybir.ActivationFunctionType.Exp

    nc = bacc.Bacc("TRN2", target_bir_lowering=False, debug=True)
    # qT2: q transposed, duplicated into both 64-partition halves.
    qT2_d = nc.dram_tensor("qT2", [128, PAIRS, S_TOT], mdt, kind="ExternalInput")
    # kT2: even k-tiles in partitions 0-63, odd k-tiles in 64-127.
    kT2_d = nc.dram_tensor("kT2", [128, PAIRS, S_TOT // 2], mdt,
                           kind="ExternalInput")
    kgT2_d = nc.dram_tensor("kgT2", [128, PAIRS, G // 2], mdt,
                            kind="ExternalInput")
    v65_d = nc.dram_tensor("v65", [128, PAIRS, NKT_LOC * NSHOT, 65], mdt,
                           kind="ExternalInput")
    vg65_d = nc.dram_tensor("vg65", [128, PAIRS, G // 128, 65], mdt,
                            kind="ExternalInput")
    oT_d = nc.dram_tensor("oT", [D, PAIRS, S_TOT], f32, kind="ExternalOutput")

    with tile.TileContext(nc) as tc:
        with (
            tc.tile_pool(name="inp", bufs=2) as inp_pool,
            tc.tile_pool(name="work", bufs=3) as work_pool,
            tc.tile_pool(name="ps_s", bufs=2, space="PSUM") as ps_pool,
            tc.tile_pool(name="ps_o", bufs=2, space="PSUM") as po_pool,
        ):
            class Unit:
                """One (pair, shot, q-chunk) attention block's emitters."""

                def __init__(self, sbufs, s, qc):
                    self.sb = sbufs
                    self.s = s
                    self.qcol = s * L + qc * QC
                    self.po = po_pool.tile([65, QC], f32, tag="po", name="po")
                    self.gt = [None] * NGROUP
                    self.ex = [None] * NGROUP

                def S_round(self, r):
                    for half in (0, 1):
                        slot = 2 * r + half
                        g, off = GROUP_OF[slot]
                        if self.gt[g] is None:
                            n = len(GROUP_SLOTS[g])
                            self.gt[g] = ps_pool.tile([128, QC * n], f32,
                                                      tag="ps", name=f"ps{g}")
                        if r < NROUND - 1:  # local rounds 0..3
                            k_lhs = self.sb["kT2"][half * 64:(half + 1) * 64,
                                                   self.s * (L // 2) + r * 128:
                                                   self.s * (L // 2) + (r + 1) * 128]
                        else:               # global round
                            k_lhs = self.sb["kgT2"][half * 64:(half + 1) * 64, :]
                        nc.tensor.matmul(
                            self.gt[g][:, off * QC:(off + 1) * QC],
                            k_lhs,
                            self.sb["qT2"][half * 64:(half + 1) * 64,
                                           self.qcol:self.qcol + QC],
                            start=True, stop=True,
                            tile_position=(half * 64, 0),
                        )

                def E(self, g):
                    n = len(GROUP_SLOTS[g])
                    expT = work_pool.tile([128, QC * n], mdt, tag="expT",
                                          name=f"expT{g}", bufs=4)
                    nc.scalar.activation(expT[:], self.gt[g][:], Exp, scale=SCALE)
                    self.ex[g] = expT

                def PV(self, g):
                    for off, slot in enumerate(GROUP_SLOTS[g]):
                        if slot < NKT_LOC:
                            v_lhs = self.sb["v65"][:, self.s * NKT_LOC + slot, :]
                        else:
                            v_lhs = self.sb["vg65"][:, slot - NKT_LOC, :]
                        nc.tensor.matmul(
                            self.po[:], v_lhs,
                            self.ex[g][:, off * QC:(off + 1) * QC],
                            start=(slot == 0), stop=(slot == NKT - 1),
                        )

                def EPI(self):
                    zsb = work_pool.tile([1, QC], f32, tag="zsb")
                    nc.vector.tensor_copy(zsb[:], self.po[64:65, :])
                    zr = work_pool.tile([1, QC], f32, tag="zr")
                    nc.vector.reciprocal_approx_fast(zr[:], zsb[:])
                    zb = work_pool.tile([64, QC], f32, tag="zb")
                    nc.gpsimd.partition_broadcast(zb[:], zr[:])
                    oT_sb = work_pool.tile([64, QC], f32, tag="oT")
                    nc.vector.tensor_mul(oT_sb[:], self.po[0:64, :], zb[:])
                    nc.sync.dma_start(
                        oT_d[:, self.sb["p"], self.qcol:self.qcol + QC], oT_sb[:])

            def load_pair(p):
                qT2_sb = inp_pool.tile([128, S_TOT], mdt, tag="qT", name="qT2_sb")
                nc.sync.dma_start(qT2_sb[:], qT2_d[:, p, :])
                kT2_sb = inp_pool.tile([128, S_TOT // 2], mdt, tag="kT",
                                       name="kT2_sb")
                nc.sync.dma_start(kT2_sb[:], kT2_d[:, p, :])
                kgT2_sb = inp_pool.tile([128, G // 2], mdt, tag="kgT",
                                        name="kgT2_sb")
                nc.sync.dma_start(kgT2_sb[:], kgT2_d[:, p, :])
                v65_sb = inp_pool.tile([128, NKT_LOC * NSHOT, 65], mdt,
                                       tag="v65", name="v65_sb")
                nc.sync.dma_start(v65_sb[:], v65_d[:, p, :, :])
                vg65_sb = inp_pool.tile([128, G // 128, 65], mdt, tag="vg65",
                                        name="vg65_sb")
                nc.sync.dma_start(vg65_sb[:], vg65_d[:, p, :, :])
                return {"p": p, "qT2": qT2_sb, "kT2": kT2_sb, "kgT2": kgT2_sb,
                        "v65": v65_sb, "vg65": vg65_sb}

            # Software-pipelined emission: unit U's last exp group, PV tail and
            # epilogue are emitted after unit U+1's first S rounds so the PE
            # always has independent work queued while ACT runs exp.
            prev = None
            sbufs = None
            for s_p in range(PAIRS):
                sbufs = load_pair(s_p)
                for s_s in range(NSHOT):
                    for s_qc in range(NQC):
                        u = Unit(sbufs, s_s, s_qc)
                        u.S_round(0)
                        u.S_round(1)
                        if prev is not None:
                            prev.E(3)
                            prev.PV(3)
                            prev.EPI()
                        u.E(0)
                        u.S_round(2)
                        u.E(1)
                        u.PV(0)
                        u.S_round(3)
                        u.PV(1)
                        u.S_round(4)
                        u.E(2)
                        u.PV(2)
                        prev = u
            prev.E(3)
            prev.PV(3)
            prev.EPI()
    nc.compile()
    _NC = nc
    return nc


def pack_inputs(q, k, v):
    """Shard + relayout full inputs into per-core input maps."""
    ndt = ml_dtypes.bfloat16 if MM_DT == "bfloat16" else np.float16
    q5 = np.ascontiguousarray(q).reshape(B, S_TOT, H, D)
    k5 = np.ascontiguousarray(k).reshape(B, S_TOT, H, D)
    v5 = np.ascontiguousarray(v).reshape(B, S_TOT, H, D)
    gidx = (np.arange(NSHOT)[:, None] * L + np.arange(PER_G)[None, :]).reshape(-1)

    in_maps = []
    for c in range(NCORES):
        qT2 = np.empty((128, PAIRS, S_TOT), ndt)
        kT2 = np.empty((128, PAIRS, S_TOT // 2), ndt)
        kgT2 = np.empty((128, PAIRS, G // 2), ndt)
        v65 = np.ones((128, PAIRS, NKT_LOC * NSHOT, 65), ndt)
        vg65 = np.ones((128, PAIRS, G // 128, 65), ndt)
        for p in range(PAIRS):
            pair = c * PAIRS + p
            b, h = divmod(pair, H)
            qT = q5[b, :, h, :].T                      # [64, S_TOT]
            qT2[0:64, p, :] = qT
            qT2[64:128, p, :] = qT
            # k tiles: [64, NSHOT, 8, 128] -> even tiles top, odd bottom
            kt = k5[b, :, h, :].T.reshape(D, NSHOT, NKT_LOC, 128)
            kT2[0:64, p, :] = kt[:, :, 0::2, :].reshape(D, S_TOT // 2)
            kT2[64:128, p, :] = kt[:, :, 1::2, :].reshape(D, S_TOT // 2)
            kg = k5[b, gidx, h, :].T                   # [64, 256]
            kgT2[0:64, p, :] = kg[:, 0:128]
            kgT2[64:128, p, :] = kg[:, 128:256]
            # [S_TOT, 64] -> [n_tiles, 128, 64] -> [128, n_tiles, 64]
            v65[:, p, :, :64] = v5[b, :, h, :].reshape(-1, 128, D).transpose(1, 0, 2)
            vg65[:, p, :, :64] = v5[b, gidx, h, :].reshape(-1, 128, D).transpose(1, 0, 2)
        in_maps.append({"qT2": qT2, "kT2": kT2, "kgT2": kgT2,
                        "v65": v65, "vg65": vg65})
    return in_maps


def unpack_outputs(results):
    """Per-core oT [D, PAIRS, S_TOT] -> full [B, S_TOT, HD]."""
    out5 = np.empty((B, S_TOT, H, D), np.float32)
    for c in range(NCORES):
        oT = results[c]["oT"]
        for p in range(PAIRS):
            b, h = divmod(c * PAIRS + p, H)
            out5[b, :, h, :] = oT[:, p, :].T
    return out5.reshape(B, S_TOT, HD)


def kernel(q, k, v, num_heads, num_shots, per_g):
    assert int(num_heads) == H and int(num_shots) == NSHOT and int(per_g) == PER_G
    nc = build_program()
    in_maps = pack_inputs(np.asarray(q), np.asarray(k), np.asarray(v))
    res = run_bass_kernel_spmd(nc, in_maps, list(range(NCORES)))
    return unpack_outputs(res.results)


# revision 14
# speedup vs baseline: 1.3954x; 1.1724x over previous
"""Sparse (shot-local + shared-global) attention on 8 Trainium2 NeuronCores.

Problem: B=2, S_TOT=4096, HD=1024 with H=16 heads (d=64), num_shots=4
(L=1024 tokens per shot), global pool = first 64 tokens of each shot
(G=256), shared by all shots of the same batch element.

Sharding: the 32 (batch, head) pairs are split 4-per-core across 8 cores
(data + head parallel). Each (b,h,shot) block is independent attention of
shape q[1024,64] against k/v[1024+256,64].

Per-core kernel (per pair, shot, 512-wide q-chunk):
  S^T[k,q]   = kT_tile.T @ qT            (PE, k tokens on partitions)
  P^T        = exp(S^T * 1/8)            (ACT, groups of 3 PSUM banks)
  [o^T; Z]   = [v | 1].T @ P^T           (PE, accumulated over k tiles)
  o^T        = o^T * (1/Z broadcast)     (DVE recip + GpSimd bcast + DVE mul)
Softmax max-subtraction is skipped: logits are ~N(0,1), |logit| < ~6, exp
is safely in range.

The S matmuls contract over d=64 (half the PE rows), so pairs of k-tiles
are packed into the two 64-row strips of the PE array (tile_position
(0,0) / (64,0)) and run concurrently: host packs even k-tiles into SBUF
partitions 0-63 and odd k-tiles into partitions 64-127, with the q tile
duplicated into both halves.

Host packs q/k into [d, tokens] (transposed) layout and v into [128, t, 65]
tiles with a ones column (the ones column makes the PV matmul emit the
softmax denominator Z as PSUM row 64). Host transposes o^T back at gather.
"""

import sys

sys.path.insert(0, "/opt/trn_rl_repo")

import ml_dtypes
import numpy as np

import concourse.bass as bass  # noqa: F401  (registers AP machinery)
import concourse.mybir as mybir
import concourse.tile as tile
from concourse import bacc
from concourse.bass_utils import run_bass_kernel_spmd

B, S_TOT, HD = 2, 4096, 1024
H, NSHOT, PER_G = 16, 4, 64
D = HD // H            # 64 head dim
L = S_TOT // NSHOT     # 1024 shot length
G = NSHOT * PER_G      # 256 global pool tokens
NCORES = 8
PAIRS = (B * H) // NCORES   # 4 (b,h) pairs per core
QC = 512                    # q chunk width (PSUM bank)
NQC = L // QC               # 2
NKT_LOC = L // 128          # 8 local k tiles per shot
NKT = NKT_LOC + G // 128    # 10 k tiles (slots) total per shot
NROUND = NKT // 2           # 5 row-packed S rounds per (shot, qc)
SCALE = 1.0 / float(np.sqrt(D))
# slot -> (exp group, offset): uniform groups of 2 slots (one S round each,
# 2 PSUM banks) so the ps pool rotates through 3 slots (pipeline depth 3)
GROUP_OF = {j: (j // 2, j % 2) for j in range(NKT)}
NGROUP = 5
GROUP_SLOTS = [[j for j in range(NKT) if GROUP_OF[j][0] == g] for g in range(NGROUP)]

MM_DT = "float16"   # matmul operand dtype ("bfloat16" | "float16")

_NC = None


def build_program():
    """Build + compile the per-core Bass program (identical on all cores)."""
    global _NC
    if _NC is not None:
        return _NC
    f32 = mybir.dt.float32
    mdt = getattr(mybir.dt, MM_DT)
    Exp = mybir.ActivationFunctionType.Exp

    nc = bacc.Bacc("TRN2", target_bir_lowering=False, debug=True)
    # qT2: q transposed, duplicated into both 64-partition halves.
    qT2_d = nc.dram_tensor("qT2", [128, PAIRS, S_TOT], mdt, kind="ExternalInput")
    # kT2: even k-tiles in partitions 0-63, odd k-tiles in 64-127.
    kT2_d = nc.dram_tensor("kT2", [128, PAIRS, S_TOT // 2], mdt,
                           kind="ExternalInput")
    kgT2_d = nc.dram_tensor("kgT2", [128, PAIRS, G // 2], mdt,
                            kind="ExternalInput")
    v65_d = nc.dram_tensor("v65", [128, PAIRS, NKT_LOC * NSHOT, 65], mdt,
                           kind="ExternalInput")
    vg65_d = nc.dram_tensor("vg65", [128, PAIRS, G // 128, 65], mdt,
                            kind="ExternalInput")
    oT_d = nc.dram_tensor("oT", [D, PAIRS, S_TOT], f32, kind="ExternalOutput")

    with tile.TileContext(nc) as tc:
        with (
            tc.tile_pool(name="inp", bufs=2) as inp_pool,
            tc.tile_pool(name="work", bufs=3) as work_pool,
            tc.tile_pool(name="ps_s", bufs=3, space="PSUM") as ps_pool,
            tc.tile_pool(name="ps_o", bufs=2, space="PSUM") as po_pool,
        ):
            class Unit:
                """One (pair, shot, q-chunk) attention block's emitters."""

                def __init__(self, sbufs, s, qc):
                    self.sb = sbufs
                    self.s = s
                    self.qcol = s * L + qc * QC
                    self.po = po_pool.tile([65, QC], f32, tag="po", name="po")
                    self.gt = [None] * NGROUP
                    self.ex = [None] * NGROUP

                def S_round(self, r):
                    for half in (0, 1):
                        slot = 2 * r + half
                        g, off = GROUP_OF[slot]
                        if self.gt[g] is None:
                            n = len(GROUP_SLOTS[g])
                            self.gt[g] = ps_pool.tile([128, QC * n], f32,
                                                      tag="ps", name=f"ps{g}")
                        if r < NROUND - 1:  # local rounds 0..3
                            k_lhs = self.sb["kT2"][half * 64:(half + 1) * 64,
                                                   self.s * (L // 2) + r * 128:
                                                   self.s * (L // 2) + (r + 1) * 128]
                        else:               # global round
                            k_lhs = self.sb["kgT2"][half * 64:(half + 1) * 64, :]
                        nc.tensor.matmul(
                            self.gt[g][:, off * QC:(off + 1) * QC],
                            k_lhs,
                            self.sb["qT2"][half * 64:(half + 1) * 64,
                                           self.qcol:self.qcol + QC],
                            start=True, stop=True,
                            tile_position=(half * 64, 0),
                        )

                def E(self, g):
                    n = len(GROUP_SLOTS[g])
                    expT = work_pool.tile([128, QC * n], mdt, tag="expT",
                                          name=f"expT{g}", bufs=4)
                    nc.scalar.activation(expT[:], self.gt[g][:], Exp, scale=SCALE)
                    self.ex[g] = expT

                def PV(self, g):
                    for off, slot in enumerate(GROUP_SLOTS[g]):
                        if slot < NKT_LOC:
                            v_lhs = self.sb["v65"][:, self.s * NKT_LOC + slot, :]
                        else:
                            v_lhs = self.sb["vg65"][:, slot - NKT_LOC, :]
                        nc.tensor.matmul(
                            self.po[:], v_lhs,
                            self.ex[g][:, off * QC:(off + 1) * QC],
                            start=(slot == 0), stop=(slot == NKT - 1),
                        )

                def EPI(self):
                    zsb = work_pool.tile([1, QC], f32, tag="zsb")
                    nc.vector.tensor_copy(zsb[:], self.po[64:65, :])
                    zr = work_pool.tile([1, QC], f32, tag="zr")
                    nc.vector.reciprocal_approx_fast(zr[:], zsb[:])
                    zb = work_pool.tile([64, QC], f32, tag="zb")
                    nc.gpsimd.partition_broadcast(zb[:], zr[:])
                    oT_sb = work_pool.tile([64, QC], f32, tag="oT")
                    nc.vector.tensor_mul(oT_sb[:], self.po[0:64, :], zb[:])
                    nc.sync.dma_start(
                        oT_d[:, self.sb["p"], self.qcol:self.qcol + QC], oT_sb[:])

            def load_pair(p):
                qT2_sb = inp_pool.tile([128, S_TOT], mdt, tag="qT", name="qT2_sb")
                nc.sync.dma_start(qT2_sb[:], qT2_d[:, p, :])
                kT2_sb = inp_pool.tile([128, S_TOT // 2], mdt, tag="kT",
                                       name="kT2_sb")
                nc.sync.dma_start(kT2_sb[:], kT2_d[:, p, :])
                kgT2_sb = inp_pool.tile([128, G // 2], mdt, tag="kgT",
                                        name="kgT2_sb")
                nc.sync.dma_start(kgT2_sb[:], kgT2_d[:, p, :])
                v65_sb = inp_pool.tile([128, NKT_LOC * NSHOT, 65], mdt,
                                       tag="v65", name="v65_sb")
                nc.sync.dma_start(v65_sb[:], v65_d[:, p, :, :])
                vg65_sb = inp_pool.tile([128, G // 128, 65], mdt, tag="vg65",
                                        name="vg65_sb")
                nc.sync.dma_start(vg65_sb[:], vg65_d[:, p, :, :])
                return {"p": p, "qT2": qT2_sb, "kT2": kT2_sb, "kgT2": kgT2_sb,
                        "v65": v65_sb, "vg65": vg65_sb}

            # Software-pipelined emission, lag-2 rotation: PV of group g
            # is emitted two (unit, group) steps after its S round + exp, so
            # the PE always has independent S work while ACT runs exp and the
            # 3-deep ps rotation absorbs the latency.
            pending = []
            sbufs = None
            for s_p in range(PAIRS):
                sbufs = load_pair(s_p)
                for s_s in range(NSHOT):
                    for s_qc in range(NQC):
                        u = Unit(sbufs, s_s, s_qc)
                        for g in range(NGROUP):
                            u.S_round(g)
                            u.E(g)
                            pending.append((u, g))
                            if len(pending) > 2:
                                pu, pg = pending.pop(0)
                                pu.PV(pg)
                                if pg == NGROUP - 1:
                                    pu.EPI()
            for pu, pg in pending:
                pu.PV(pg)
                if pg == NGROUP - 1:
                    pu.EPI()
    nc.compile()
    _NC = nc
    return nc


def pack_inputs(q, k, v):
    """Shard + relayout full inputs into per-core input maps."""
    ndt = ml_dtypes.bfloat16 if MM_DT == "bfloat16" else np.float16
    q5 = np.ascontiguousarray(q).reshape(B, S_TOT, H, D)
    k5 = np.ascontiguousarray(k).reshape(B, S_TOT, H, D)
    v5 = np.ascontiguousarray(v).reshape(B, S_TOT, H, D)
    gidx = (np.arange(NSHOT)[:, None] * L + np.arange(PER_G)[None, :]).reshape(-1)

    in_maps = []
    for c in range(NCORES):
        qT2 = np.empty((128, PAIRS, S_TOT), ndt)
        kT2 = np.empty((128, PAIRS, S_TOT // 2), ndt)
        kgT2 = np.empty((128, PAIRS, G // 2), ndt)
        v65 = np.ones((128, PAIRS, NKT_LOC * NSHOT, 65), ndt)
        vg65 = np.ones((128, PAIRS, G // 128, 65), ndt)
        for p in range(PAIRS):
            pair = c * PAIRS + p
            b, h = divmod(pair, H)
            qT = q5[b, :, h, :].T                      # [64, S_TOT]
            qT2[0:64, p, :] = qT
            qT2[64:128, p, :] = qT
            # k tiles: [64, NSHOT, 8, 128] -> even tiles top, odd bottom
            kt = k5[b, :, h, :].T.reshape(D, NSHOT, NKT_LOC, 128)
            kT2[0:64, p, :] = kt[:, :, 0::2, :].reshape(D, S_TOT // 2)
            kT2[64:128, p, :] = kt[:, :, 1::2, :].reshape(D, S_TOT // 2)
            kg = k5[b, gidx, h, :].T                   # [64, 256]
            kgT2[0:64, p, :] = kg[:, 0:128]
            kgT2[64:128, p, :] = kg[:, 128:256]
            # [S_TOT, 64] -> [n_tiles, 128, 64] -> [128, n_tiles, 64]
            v65[:, p, :, :64] = v5[b, :, h, :].reshape(-1, 128, D).transpose(1, 0, 2)
            vg65[:, p, :, :64] = v5[b, gidx, h, :].reshape(-1, 128, D).transpose(1, 0, 2)
        in_maps.append({"qT2": qT2, "kT2": kT2, "kgT2": kgT2,
                        "v65": v65, "vg65": vg65})
    return in_maps


def unpack_outputs(results):
    """Per-core oT [D, PAIRS, S_TOT] -> full [B, S_TOT, HD]."""
    out5 = np.empty((B, S_TOT, H, D), np.float32)
    for c in range(NCORES):
        oT = results[c]["oT"]
        for p in range(PAIRS):
            b, h = divmod(c * PAIRS + p, H)
            out5[b, :, h, :] = oT[:, p, :].T
    return out5.reshape(B, S_TOT, HD)


def kernel(q, k, v, num_heads, num_shots, per_g):
    assert int(num_heads) == H and int(num_shots) == NSHOT and int(per_g) == PER_G
    nc = build_program()
    in_maps = pack_inputs(np.asarray(q), np.asarray(k), np.asarray(v))
    res = run_bass_kernel_spmd(nc, in_maps, list(range(NCORES)))
    return unpack_outputs(res.results)


# revision 15
# speedup vs baseline: 1.4113x; 1.0114x over previous
"""Sparse (shot-local + shared-global) attention on 8 Trainium2 NeuronCores.

Problem: B=2, S_TOT=4096, HD=1024 with H=16 heads (d=64), num_shots=4
(L=1024 tokens per shot), global pool = first 64 tokens of each shot
(G=256), shared by all shots of the same batch element.

Sharding: the 32 (batch, head) pairs are split 4-per-core across 8 cores
(data + head parallel). Each (b,h,shot) block is independent attention of
shape q[1024,64] against k/v[1024+256,64].

Per-core kernel (per pair, shot, 512-wide q-chunk):
  S^T[k,q]   = kT_tile.T @ qT            (PE, k tokens on partitions)
  P^T        = exp(S^T * 1/8)            (ACT, groups of 2 PSUM banks)
  [o^T; Z]   = [v | 1].T @ P^T           (PE, accumulated over k tiles)
  o^T        = o^T * (1/Z broadcast)     (DVE recip + GpSimd bcast + DVE mul)
Softmax max-subtraction is skipped: logits are ~N(0,1), |logit| < ~6, exp
is safely in range.

The S matmuls contract over d=64 (half the PE rows), so pairs of k-tiles
are packed into the two 64-row strips of the PE array (tile_position
(0,0) / (64,0)) and run concurrently: host packs even k-tiles into SBUF
partitions 0-63 and odd k-tiles into partitions 64-127, with the q tile
duplicated into both halves.

Host packs q/k into [d, tokens] (transposed) layout and v into [128, t, 65]
tiles with a ones column (the ones column makes the PV matmul emit the
softmax denominator Z as PSUM row 64). Host transposes o^T back at gather.
"""

import sys

sys.path.insert(0, "/opt/trn_rl_repo")

import ml_dtypes
import numpy as np

import concourse.bass as bass  # noqa: F401  (registers AP machinery)
import concourse.mybir as mybir
import concourse.tile as tile
from concourse import bacc
from concourse.bass_utils import run_bass_kernel_spmd

B, S_TOT, HD = 2, 4096, 1024
H, NSHOT, PER_G = 16, 4, 64
D = HD // H            # 64 head dim
L = S_TOT // NSHOT     # 1024 shot length
G = NSHOT * PER_G      # 256 global pool tokens
NCORES = 8
PAIRS = (B * H) // NCORES   # 4 (b,h) pairs per core
QC = 512                    # q chunk width (PSUM bank)
NQC = L // QC               # 2
NKT_LOC = L // 128          # 8 local k tiles per shot
NKT = NKT_LOC + G // 128    # 10 k tiles (slots) total per shot
NROUND = NKT // 2           # 5 row-packed S rounds per (shot, qc)
SCALE = 1.0 / float(np.sqrt(D))
# slot -> (exp group, offset): uniform groups of 2 slots (one S round each,
# 2 PSUM banks) so the ps pool rotates through 3 slots (pipeline depth 3)
GROUP_OF = {j: (j // 2, j % 2) for j in range(NKT)}
NGROUP = 5
GROUP_SLOTS = [[j for j in range(NKT) if GROUP_OF[j][0] == g] for g in range(NGROUP)]

MM_DT = "float16"   # matmul operand dtype ("bfloat16" | "float16")

_NC = None


def build_program():
    """Build + compile the per-core Bass program (identical on all cores)."""
    global _NC
    if _NC is not None:
        return _NC
    f32 = mybir.dt.float32
    mdt = getattr(mybir.dt, MM_DT)
    Exp = mybir.ActivationFunctionType.Exp

    nc = bacc.Bacc("TRN2", target_bir_lowering=False, debug=True)
    # qT2: q transposed, duplicated into both 64-partition halves.
    qT2_d = nc.dram_tensor("qT2", [128, PAIRS, S_TOT], mdt, kind="ExternalInput")
    # kT2: even k-tiles in partitions 0-63, odd k-tiles in 64-127.
    kT2_d = nc.dram_tensor("kT2", [128, PAIRS, S_TOT // 2], mdt,
                           kind="ExternalInput")
    kgT2_d = nc.dram_tensor("kgT2", [128, PAIRS, G // 2], mdt,
                            kind="ExternalInput")
    v65_d = nc.dram_tensor("v65", [128, PAIRS, NKT_LOC * NSHOT, 65], mdt,
                           kind="ExternalInput")
    vg65_d = nc.dram_tensor("vg65", [128, PAIRS, G // 128, 65], mdt,
                            kind="ExternalInput")
    oT_d = nc.dram_tensor("oT", [D, PAIRS, S_TOT], f32, kind="ExternalOutput")

    with tile.TileContext(nc) as tc:
        with (
            tc.tile_pool(name="inp", bufs=2) as inp_pool,
            tc.tile_pool(name="work", bufs=3) as work_pool,
            tc.tile_pool(name="ps_s", bufs=3, space="PSUM") as ps_pool,
            tc.tile_pool(name="ps_o", bufs=2, space="PSUM") as po_pool,
        ):
            class Unit:
                """One (pair, shot, q-chunk) attention block's emitters."""

                def __init__(self, sbufs, s, qc):
                    self.sb = sbufs
                    self.s = s
                    self.qcol = s * L + qc * QC
                    self.po = po_pool.tile([65, QC], f32, tag="po", name="po")
                    self.gt = [None] * NGROUP
                    self.ex = [None] * NGROUP

                def S_round(self, r):
                    for half in (0, 1):
                        slot = 2 * r + half
                        g, off = GROUP_OF[slot]
                        if self.gt[g] is None:
                            n = len(GROUP_SLOTS[g])
                            self.gt[g] = ps_pool.tile([128, QC * n], f32,
                                                      tag="ps", name=f"ps{g}")
                        if r < NROUND - 1:  # local rounds 0..3
                            k_lhs = self.sb["kT2"][half * 64:(half + 1) * 64,
                                                   self.s * (L // 2) + r * 128:
                                                   self.s * (L // 2) + (r + 1) * 128]
                        else:               # global round
                            k_lhs = self.sb["kgT2"][half * 64:(half + 1) * 64, :]
                        nc.tensor.matmul(
                            self.gt[g][:, off * QC:(off + 1) * QC],
                            k_lhs,
                            self.sb["qT2"][half * 64:(half + 1) * 64,
                                           self.qcol:self.qcol + QC],
                            start=True, stop=True,
                            tile_position=(half * 64, 0),
                        )

                def E(self, g):
                    n = len(GROUP_SLOTS[g])
                    expT = work_pool.tile([128, QC * n], mdt, tag="expT",
                                          name=f"expT{g}", bufs=4)
                    nc.scalar.activation(expT[:], self.gt[g][:], Exp, scale=SCALE)
                    self.ex[g] = expT

                def PV(self, g):
                    for off, slot in enumerate(GROUP_SLOTS[g]):
                        if slot < NKT_LOC:
                            v_lhs = self.sb["v65"][:, self.s * NKT_LOC + slot, :]
                        else:
                            v_lhs = self.sb["vg65"][:, slot - NKT_LOC, :]
                        nc.tensor.matmul(
                            self.po[:], v_lhs,
                            self.ex[g][:, off * QC:(off + 1) * QC],
                            start=(slot == 0), stop=(slot == NKT - 1),
                        )

                def EPI(self):
                    zsb = work_pool.tile([1, QC], f32, tag="zsb")
                    nc.vector.tensor_copy(zsb[:], self.po[64:65, :])
                    zr = work_pool.tile([1, QC], f32, tag="zr")
                    nc.vector.reciprocal_approx_fast(zr[:], zsb[:])
                    zb = work_pool.tile([64, QC], f32, tag="zb")
                    nc.gpsimd.partition_broadcast(zb[:], zr[:])
                    oT_sb = work_pool.tile([64, QC], f32, tag="oT")
                    nc.vector.tensor_mul(oT_sb[:], self.po[0:64, :], zb[:])
                    nc.sync.dma_start(
                        oT_d[:, self.sb["p"], self.qcol:self.qcol + QC], oT_sb[:])

            def load_pair(p):
                qT2_sb = inp_pool.tile([128, S_TOT], mdt, tag="qT", name="qT2_sb")
                nc.sync.dma_start(qT2_sb[:, :L], qT2_d[:, p, :L])
                kT2_sb = inp_pool.tile([128, S_TOT // 2], mdt, tag="kT",
                                       name="kT2_sb")
                nc.sync.dma_start(kT2_sb[:, :L // 2], kT2_d[:, p, :L // 2])
                nc.sync.dma_start(qT2_sb[:, L:], qT2_d[:, p, L:])
                nc.sync.dma_start(kT2_sb[:, L // 2:], kT2_d[:, p, L // 2:])
                kgT2_sb = inp_pool.tile([128, G // 2], mdt, tag="kgT",
                                        name="kgT2_sb")
                nc.sync.dma_start(kgT2_sb[:], kgT2_d[:, p, :])
                v65_sb = inp_pool.tile([128, NKT_LOC * NSHOT, 65], mdt,
                                       tag="v65", name="v65_sb")
                nc.sync.dma_start(v65_sb[:], v65_d[:, p, :, :])
                vg65_sb = inp_pool.tile([128, G // 128, 65], mdt, tag="vg65",
                                        name="vg65_sb")
                nc.sync.dma_start(vg65_sb[:], vg65_d[:, p, :, :])
                return {"p": p, "qT2": qT2_sb, "kT2": kT2_sb, "kgT2": kgT2_sb,
                        "v65": v65_sb, "vg65": vg65_sb}

            # Software-pipelined emission, lag-2 rotation: PV of group g
            # is emitted two (unit, group) steps after its S round + exp, so
            # the PE always has independent S work while ACT runs exp and the
            # 3-deep ps rotation absorbs the latency.
            pending = []
            sbufs = None
            for s_p in range(PAIRS):
                sbufs = load_pair(s_p)
                for s_s in range(NSHOT):
                    for s_qc in range(NQC):
                        u = Unit(sbufs, s_s, s_qc)
                        for g in range(NGROUP):
                            u.S_round(g)
                            u.E(g)
                            pending.append((u, g))
                            if len(pending) > 2:
                                pu, pg = pending.pop(0)
                                pu.PV(pg)
                                if pg == NGROUP - 1:
                                    pu.EPI()
            for pu, pg in pending:
                pu.PV(pg)
                if pg == NGROUP - 1:
                    pu.EPI()
    nc.compile()
    _NC = nc
    return nc


def pack_inputs(q, k, v):
    """Shard + relayout full inputs into per-core input maps."""
    ndt = ml_dtypes.bfloat16 if MM_DT == "bfloat16" else np.float16
    q5 = np.ascontiguousarray(q).reshape(B, S_TOT, H, D)
    k5 = np.ascontiguousarray(k).reshape(B, S_TOT, H, D)
    v5 = np.ascontiguousarray(v).reshape(B, S_TOT, H, D)
    gidx = (np.arange(NSHOT)[:, None] * L + np.arange(PER_G)[None, :]).reshape(-1)

    in_maps = []
    for c in range(NCORES):
        qT2 = np.empty((128, PAIRS, S_TOT), ndt)
        kT2 = np.empty((128, PAIRS, S_TOT // 2), ndt)
        kgT2 = np.empty((128, PAIRS, G // 2), ndt)
        v65 = np.ones((128, PAIRS, NKT_LOC * NSHOT, 65), ndt)
        vg65 = np.ones((128, PAIRS, G // 128, 65), ndt)
        for p in range(PAIRS):
            pair = c * PAIRS + p
            b, h = divmod(pair, H)
            qT = q5[b, :, h, :].T                      # [64, S_TOT]
            qT2[0:64, p, :] = qT
            qT2[64:128, p, :] = qT
            # k tiles: [64, NSHOT, 8, 128] -> even tiles top, odd bottom
            kt = k5[b, :, h, :].T.reshape(D, NSHOT, NKT_LOC, 128)
            kT2[0:64, p, :] = kt[:, :, 0::2, :].reshape(D, S_TOT // 2)
            kT2[64:128, p, :] = kt[:, :, 1::2, :].reshape(D, S_TOT // 2)
            kg = k5[b, gidx, h, :].T                   # [64, 256]
            kgT2[0:64, p, :] = kg[:, 0:128]
            kgT2[64:128, p, :] = kg[:, 128:256]
            # [S_TOT, 64] -> [n_tiles, 128, 64] -> [128, n_tiles, 64]
            v65[:, p, :, :64] = v5[b, :, h, :].reshape(-1, 128, D).transpose(1, 0, 2)
            vg65[:, p, :, :64] = v5[b, gidx, h, :].reshape(-1, 128, D).transpose(1, 0, 2)
        in_maps.append({"qT2": qT2, "kT2": kT2, "kgT2": kgT2,
                        "v65": v65, "vg65": vg65})
    return in_maps


def unpack_outputs(results):
    """Per-core oT [D, PAIRS, S_TOT] -> full [B, S_TOT, HD]."""
    out5 = np.empty((B, S_TOT, H, D), np.float32)
    for c in range(NCORES):
        oT = results[c]["oT"]
        for p in range(PAIRS):
            b, h = divmod(c * PAIRS + p, H)
            out5[b, :, h, :] = oT[:, p, :].T
    return out5.reshape(B, S_TOT, HD)


def kernel(q, k, v, num_heads, num_shots, per_g):
    assert int(num_heads) == H and int(num_shots) == NSHOT and int(per_g) == PER_G
    nc = build_program()
    in_maps = pack_inputs(np.asarray(q), np.asarray(k), np.asarray(v))
    res = run_bass_kernel_spmd(nc, in_maps, list(range(NCORES)))
    return unpack_outputs(res.results)
